# revision 1
# baseline (speedup 1.0000x reference)
"""Single-head attention (B=4, T=4096, C=1024, H=64) on 8 trn2 NeuronCores.

Sharding: 8 shards = (batch b, query-half h).  Each core receives x[b]
pre-transposed to xT [C=1024, T=4096]; for h==1 the T columns are rotated by
2048 so that "this core's" 2048 queries are always columns 0:2048 (softmax is
permutation-invariant over keys, so rotating the key order changes nothing).
This keeps the SPMD program identical on every core with no rank logic.

Per-core kernel (flash-attention style; the [T,T] score matrix never touches
DRAM):
  phase 1: stream xT in [128,512] tiles; PE computes KVT = [Wk|Wv]^T x^T
           ([128,4096], rows 0:64 = K^T, 64:128 = V^T) and Q^T [64,2048]
           (f32r matmuls, contraction over C in 8 chunks of 128);
           V^T tiles are PE-transposed back to V [s,64] and a ones column is
           appended (-> softmax denominator comes out of the attn@V matmul).
  phase 2: for each 512-wide query chunk: for each 128-key tile,
           PE: scoresT[s=128, t=512] = K_tile^T{64,128}.T @ Q^T{64,512}
           ACT: exp(0.125 * scoresT) -> SBUF   (scores are O(3), no max-sub
           needed for a numerically safe softmax)
           PE: outT[65,512] += V_aug[s,65].T @ exp  (accumulate over 32 tiles)
           then PE-transpose outT back to [t,65], multiply rows by the
           reciprocal of column 64 (the exp-sum), DMA out.
"""

import os
import sys

for _p in ("/opt/trn_rl_repo", "/root/.axon_site/_ro/trn_rl_repo"):
    if os.path.isdir(_p) and _p not in sys.path:
        sys.path.append(_p)

import numpy as np

import concourse.bacc as bacc
import concourse.mybir as mybir
import concourse.tile as tile
from concourse.bass_utils import run_bass_kernel_spmd
from concourse.masks import make_identity

B = 4
T = 4096
C = 1024
H = 64
TQ = T // 2  # queries per core
N_CORES = 8

F32 = mybir.dt.float32
F32R = mybir.dt.float32r

NC_CH = C // 128  # 8 contraction chunks
NSB = T // 512  # 8 key/source blocks of 512
NST = T // 128  # 32 key tiles of 128
NTC = TQ // 512  # 4 query chunks of 512


def _build_module():
    nc = bacc.Bacc("TRN2", target_bir_lowering=False, debug=False, num_devices=N_CORES)

    xT = nc.dram_tensor("xT", [NSB, NC_CH, 128, 512], F32, kind="ExternalInput").ap()
    wkpad = nc.dram_tensor("wkpad", [NC_CH, 128, 2 * H], F32, kind="ExternalInput").ap()
    wqv = nc.dram_tensor("wqv", [NC_CH, 128, 2 * H], F32, kind="ExternalInput").ap()
    out = nc.dram_tensor("out", [TQ, H], F32, kind="ExternalOutput").ap()

    EXP = mybir.ActivationFunctionType.Exp

    with tile.TileContext(nc) as tc:
        with (
            tc.tile_pool(name="const", bufs=1) as const_pool,
            tc.tile_pool(name="xt", bufs=32) as xt_pool,
            tc.tile_pool(name="big", bufs=1) as big_pool,
            tc.tile_pool(name="exp", bufs=6) as exp_pool,
            tc.tile_pool(name="outts", bufs=2) as outts_pool,
            tc.tile_pool(name="small", bufs=4) as small_pool,
            tc.tile_pool(name="p1", bufs=2, space="PSUM") as psum_p1,
            tc.tile_pool(name="psc", bufs=2, space="PSUM") as psum_sc,
            tc.tile_pool(name="pacc", bufs=2, space="PSUM") as psum_acc,
        ):
            # ---- constants ----
            wkpad_sb = const_pool.tile([128, NC_CH, 2 * H], F32R, tag="wkpad")
            wqv_sb = const_pool.tile([128, NC_CH, 2 * H], F32R, tag="wqv")
            ident_f32 = const_pool.tile([128, 128], F32, tag="ident_f32")
            ones_f32 = const_pool.tile([128, NST, 1], F32, tag="ones")
            for c in range(NC_CH):
                nc.sync.dma_start(wkpad_sb[:, c, :], wkpad[c].bitcast(F32R))
                nc.sync.dma_start(wqv_sb[:, c, :], wqv[c].bitcast(F32R))
            make_identity(nc, ident_f32[:])
            nc.gpsimd.memset(ones_f32[:], 1.0)

            # ---- persistent activations ----
            kt_sb = big_pool.tile([128, T], F32R, tag="kt")  # K^T, rows 64: = 0
            qv_sb = big_pool.tile([128, TQ], F32R, tag="qv")  # Q^T | V^T(head)
            vt_f32 = big_pool.tile([128, T], F32, tag="vtf32")  # V^T in rows 64:
            va = big_pool.tile([128, NST, 66], F32R, tag="va")  # V_aug per s-tile
            nc.vector.tensor_copy(va[:, :, 64:65], ones_f32[:])

            # ---- phase 1: projections (emitted in two halves, with phase-2
            # score work interleaved so the scheduler overlaps it with the
            # second half of the x DMA stream) ----
            dma_engines = (nc.sync, nc.gpsimd, nc.scalar)

            def emit_proj_block(sb):
                    # contiguous [128,512] tiles; triggers rotate across three
                    # sequencers (a dma_start costs ~650 ns serially on its
                    # issuing sequencer)
                    xts = []
                    for c in range(NC_CH):
                        xt = xt_pool.tile([128, 512], F32R, tag="xt")
                        if sb < 2:
                            # halve the first blocks so all 16 queues work on
                            # them at once -> earliest possible first matmul
                            dma_engines[c % 3].dma_start(
                                xt[:, 0:256], xT[sb, c, :, 0:256].bitcast(F32R)
                            )
                            dma_engines[(c + 1) % 3].dma_start(
                                xt[:, 256:512], xT[sb, c, :, 256:512].bitcast(F32R)
                            )
                        else:
                            dma_engines[c % 3].dma_start(xt[:], xT[sb, c].bitcast(F32R))
                        xts.append(xt)
                    kt_ps = psum_p1.tile([128, 512], F32, tag="p1")
                    for c in range(NC_CH):
                        nc.tensor.matmul(
                            kt_ps[:],
                            wkpad_sb[:, c, :],
                            xts[c][:],
                            start=(c == 0),
                            stop=(c == NC_CH - 1),
                        )
                    nc.vector.tensor_copy(kt_sb[:, sb * 512 : (sb + 1) * 512], kt_ps[:])
                    qv_ps = psum_p1.tile([128, 512], F32, tag="p1")
                    for c in range(NC_CH):
                        nc.tensor.matmul(
                            qv_ps[:],
                            wqv_sb[:, c, :],
                            xts[c][:],
                            start=(c == 0),
                            stop=(c == NC_CH - 1),
                        )
                    nc.vector.tensor_copy(
                        vt_f32[64:128, sb * 512 : (sb + 1) * 512], qv_ps[64:128, :]
                    )
                    if sb < NTC:  # query half: keep Q^T (rows 64: are V^T, benign)
                        nc.vector.tensor_copy(qv_sb[:, sb * 512 : (sb + 1) * 512], qv_ps[:])
                    for j in range(4):  # V tiles of this block
                        st = sb * 4 + j
                        vt_ps = psum_p1.tile([128, 64], F32, tag="p1")
                        nc.tensor.transpose(
                            vt_ps[:],
                            vt_f32[64:128, st * 128 : (st + 1) * 128],
                            ident_f32[64:128, 64:128],
                        )
                        nc.vector.tensor_copy(va[:, st, 0:64], vt_ps[:])


            # ---- phase 2: attention, two query chunks (1024 queries) at a time ----
            outt_tiles = {}

            def emit_attn(tcp, st_lo, st_hi):
                tc0 = 2 * tcp
                if tcp not in outt_tiles:
                    oa = psum_acc.tile([65, 512], F32, tag="acc", name=f"outt_a{tcp}")
                    ob = psum_acc.tile([65, 512], F32, tag="acc", name=f"outt_b{tcp}")
                    outt_tiles[tcp] = (oa, ob)
                outt_a, outt_b = outt_tiles[tcp]
                for st in range(st_lo, st_hi):
                    kt_slice = kt_sb[:, st * 128 : (st + 1) * 128]
                    sc_ps = psum_sc.tile([128, 1024], F32, tag="sc")
                    for i in range(2):
                        nc.tensor.matmul(
                            sc_ps[:, i * 512 : (i + 1) * 512],
                            kt_slice,
                            qv_sb[:, (tc0 + i) * 512 : (tc0 + i + 1) * 512],
                            start=True,
                            stop=True,
                        )
                    ex = exp_pool.tile([128, 1024], F32R, tag="exp")
                    nc.scalar.activation(ex[:], sc_ps[:], EXP, scale=0.125)
                    for i, outt_ps in enumerate((outt_a, outt_b)):
                        nc.tensor.matmul(
                            outt_ps[:],
                            va[:, st, 0:65],
                            ex[:, i * 512 : (i + 1) * 512],
                            start=(st == 0),
                            stop=(st == NST - 1),
                        )

            def emit_epilogue(tcp):
                tc0 = 2 * tcp
                for i, outt_ps in enumerate(outt_tiles[tcp]):
                    tci = tc0 + i
                    outt_sb = outts_pool.tile([65, 512], F32, tag="outts")
                    nc.vector.tensor_copy(outt_sb[:], outt_ps[:])
                    for k in range(4):
                        o_ps = psum_p1.tile([128, 65], F32, tag="p1")
                        nc.tensor.transpose(
                            o_ps[:], outt_sb[:, k * 128 : (k + 1) * 128], ident_f32[0:65, 0:65]
                        )
                        rc = small_pool.tile([128, 1], F32, tag="rc")
                        nc.vector.reciprocal(rc[:], o_ps[:, 64:65])
                        o_sb = small_pool.tile([128, H], F32, tag="osb")
                        nc.vector.tensor_scalar_mul(o_sb[:], o_ps[:, 0:H], rc[:])
                        row = tci * 512 + k * 128
                        nc.sync.dma_start(out[row : row + 128, :], o_sb[:])

            # emission order: first half of projections; then phase-2 scores
            # over the ready key tiles (they overlap the second DMA half);
            # then the rest, pipelined.
            for sb in range(NTC):
                emit_proj_block(sb)
            emit_attn(0, 0, 16)
            for sb in range(NTC, NSB):
                emit_proj_block(sb)
            emit_attn(0, 16, NST)
            emit_attn(1, 0, 16)
            emit_epilogue(0)
            emit_attn(1, 16, NST)
            emit_epilogue(1)

    nc.compile()
    return nc


_NC_CACHE = None


def _get_module():
    global _NC_CACHE
    if _NC_CACHE is None:
        _NC_CACHE = _build_module()
    return _NC_CACHE


def _make_in_maps(x, Wq, Wk, Wv):
    xT = np.transpose(np.asarray(x, dtype=np.float32), (0, 2, 1))  # [B, C, T]
    # pre-tile for contiguous 256 KiB DMAs: [C,T] -> [NSB, NC_CH, 128, 512]
    wq = np.asarray(Wq, dtype=np.float32)
    wk = np.asarray(Wk, dtype=np.float32)
    wv = np.asarray(Wv, dtype=np.float32)
    # [Wk | 0]: scores contraction zero-padded to K=128 (f32r matmuls run at
    # half rate for K=64); [Wq | Wv]: the V^T rows double as finite padding
    # rows on the rhs side of the scores matmul.
    wkpad = np.ascontiguousarray(
        np.concatenate([wk, np.zeros_like(wk)], axis=1).reshape(NC_CH, 128, 2 * H)
    )
    wqv = np.ascontiguousarray(
        np.concatenate([wq, wv], axis=1).reshape(NC_CH, 128, 2 * H)
    )
    in_maps = []
    for core in range(N_CORES):
        b, h = divmod(core, 2)
        xt = xT[b]
        if h == 1:
            xt = np.concatenate([xt[:, TQ:], xt[:, :TQ]], axis=1)
        xt = np.ascontiguousarray(
            xt.reshape(NC_CH, 128, NSB, 512).transpose(2, 0, 1, 3)
        )
        in_maps.append({"xT": xt, "wkpad": wkpad, "wqv": wqv})
    return in_maps


def run(x, Wq, Wk, Wv, **spmd_kwargs):
    """Run on hardware; returns (output, BassKernelResults)."""
    nc = _get_module()
    in_maps = _make_in_maps(x, Wq, Wk, Wv)
    res = run_bass_kernel_spmd(nc, in_maps, core_ids=list(range(N_CORES)), **spmd_kwargs)
    out = np.empty((B, T, H), dtype=np.float32)
    for core in range(N_CORES):
        b, h = divmod(core, 2)
        out[b, h * TQ : (h + 1) * TQ, :] = res.results[core]["out"]
    return out, res


def kernel(x, Wq, Wk, Wv):
    out, _ = run(x, Wq, Wk, Wv)
    return out



# revision 8
# speedup vs baseline: 1.2388x; 1.2388x over previous
"""Single-head attention (B=4, T=4096, C=1024, H=64) on 8 trn2 NeuronCores.

Sharding: 8 shards = (batch b, query-half h).  Each core receives x[b]
pre-transposed to xT [C=1024, T=4096] in bf16; for h==1 the T columns are
rotated by 2048 so that "this core's" 2048 queries are always columns 0:2048
(softmax is permutation-invariant over keys).  SPMD program identical on all
cores, no rank logic.

Per-core kernel (flash-attention style; [T,T] scores never touch DRAM):
  phase 1: stream xT bf16 in [128, 4x512] half-block DMAs; PE computes
           KV^T = [Wk|Wv]^T x^T  ([128,4096]: rows 0:64 K^T, 64:128 V^T)
           over all 8 blocks, and Q^T = Wq^T x^T ([64,2048]) over the 4
           query blocks only; V^T tiles are PE-transposed back to V[s,64]
           (bf16) with a ones column appended (softmax denominator falls
           out of the attn@V matmul).
  phase 2: for each 128-key tile st and 1024-query group: PE scores^T
           [128,1024] = K_tile^T{64,128}.T @ Q^T{64,1024} (K=64 contraction,
           bf16); ACT exp(0.125 s) -> bf16 SBUF; PE outT[65,:] += V_aug.T @ ex
           accumulated over 32 tiles; PE-transpose outT, DVE row-scale by
           the reciprocal of the exp-sum column, DMA out (f32).
Emission interleaves phase 2 under phase 1 so ACT starts early and PE
never starves (PE p-state needs continuous execution to reach 2.4 GHz).
"""

import os
import sys

for _p in ("/opt/trn_rl_repo", "/root/.axon_site/_ro/trn_rl_repo"):
    if os.path.isdir(_p) and _p not in sys.path:
        sys.path.append(_p)

import numpy as np

import concourse.bacc as bacc
import concourse.mybir as mybir
import concourse.tile as tile
from concourse.bass_utils import run_bass_kernel_spmd
from concourse.masks import make_identity

B = 4
T = 4096
C = 1024
H = 64
TQ = T // 2  # queries per core
N_CORES = 8

F32 = mybir.dt.float32
BF16 = mybir.dt.bfloat16

NC_CH = C // 128  # 8 contraction chunks
NSB = T // 512  # 8 key/source blocks of 512
NST = T // 128  # 32 key tiles of 128
NQB = TQ // 512  # 4 query blocks/chunks of 512


def _build_module():
    nc = bacc.Bacc("TRN2", target_bir_lowering=False, debug=False, num_devices=N_CORES)

    # xt: [16 half-blocks, 128, 4, 512] bf16; half-block g=(sb,half) holds
    # c-chunks 4*half..4*half+3 of block sb, contiguous per partition.
    xt_d = nc.dram_tensor("xt", [2 * NSB, 128, 4, 512], BF16, kind="ExternalInput").ap()
    # wkv: [128, 8 cchunks, 128] bf16 ([:, c, 0:64]=Wk, [:, c, 64:128]=Wv)
    wkv_d = nc.dram_tensor("wkv", [128, NC_CH, 128], BF16, kind="ExternalInput").ap()
    # wq: [128, 8 cchunks, 64] bf16
    wq_d = nc.dram_tensor("wq", [128, NC_CH, 64], BF16, kind="ExternalInput").ap()
    out = nc.dram_tensor("out", [TQ, H], F32, kind="ExternalOutput").ap()

    EXP = mybir.ActivationFunctionType.Exp

    with tile.TileContext(nc) as tc:
        with (
            tc.tile_pool(name="const", bufs=1) as const_pool,
            tc.tile_pool(name="xt", bufs=3) as xt_pool,
            tc.tile_pool(name="big", bufs=1) as big_pool,
            tc.tile_pool(name="vstage", bufs=2) as vstage_pool,
            tc.tile_pool(name="exp", bufs=5) as exp_pool,
            tc.tile_pool(name="outts", bufs=2) as outts_pool,
            tc.tile_pool(name="small", bufs=4) as small_pool,
            tc.tile_pool(name="p1", bufs=2, space="PSUM") as psum_p1,
            tc.tile_pool(name="psc", bufs=2, space="PSUM") as psum_sc,
            tc.tile_pool(name="pacc", bufs=2, space="PSUM") as psum_acc,
        ):
            # ---- constants ----
            wkv_sb = const_pool.tile([128, NC_CH, 128], BF16, tag="wkv")
            wq_sb = const_pool.tile([128, NC_CH, 64], BF16, tag="wq")
            ident_bf = const_pool.tile([128, 128], BF16, tag="ident_bf")
            ident_f32 = const_pool.tile([128, 128], F32, tag="ident_f32")
            nc.sync.dma_start(wkv_sb[:], wkv_d)
            nc.sync.dma_start(wq_sb[:], wq_d)
            make_identity(nc, ident_bf[:])
            make_identity(nc, ident_f32[:])

            # ---- persistent activations ----
            kt_sb = big_pool.tile([64, T], BF16, tag="kt")  # K^T
            qt_sb = big_pool.tile([64, TQ], BF16, tag="qt")  # Q^T
            va = big_pool.tile([128, NST, 66], BF16, tag="va")  # V_aug per s-tile
            nc.gpsimd.memset(va[:, :, 64:65], 1.0)

            dma_engines = (nc.sync, nc.gpsimd, nc.scalar)
            dma_i = [0]

            def next_dma():
                e = dma_engines[dma_i[0] % len(dma_engines)]
                dma_i[0] += 1
                return e

            # ---- phase 1: projections, one block of 512 keys at a time ----
            def emit_proj_block(sb):
                xt = xt_pool.tile([128, NC_CH, 512], BF16, tag="xt")
                for half in range(2):
                    next_dma().dma_start(
                        xt[:, 4 * half : 4 * half + 4, :], xt_d[2 * sb + half]
                    )
                kv_ps = psum_p1.tile([128, 512], F32, tag="p1")
                for c in range(NC_CH):
                    nc.tensor.matmul(
                        kv_ps[:],
                        wkv_sb[:, c, :],
                        xt[:, c, :],
                        start=(c == 0),
                        stop=(c == NC_CH - 1),
                    )
                nc.vector.tensor_copy(kt_sb[:, sb * 512 : (sb + 1) * 512], kv_ps[0:64, :])
                vt_sb = vstage_pool.tile([128, 512], BF16, tag="vst")
                nc.vector.tensor_copy(vt_sb[64:128, :], kv_ps[64:128, :])
                if sb < NQB:  # query half: also compute Q^T for these columns
                    q_ps = psum_p1.tile([64, 512], F32, tag="p1")
                    for c in range(NC_CH):
                        nc.tensor.matmul(
                            q_ps[:],
                            wq_sb[:, c, :],
                            xt[:, c, :],
                            start=(c == 0),
                            stop=(c == NC_CH - 1),
                        )
                    nc.vector.tensor_copy(qt_sb[:, sb * 512 : (sb + 1) * 512], q_ps[:])
                for j in range(4):  # V tiles of this block
                    st = sb * 4 + j
                    vt_ps = psum_p1.tile([128, 64], BF16, tag="p1")
                    nc.tensor.transpose(
                        vt_ps[:],
                        vt_sb[64:128, j * 128 : (j + 1) * 128],
                        ident_bf[64:128, 64:128],
                    )
                    nc.vector.tensor_copy(va[:, st, 0:64], vt_ps[:])

            # ---- phase 2: attention, 1024 queries (one tcp) at a time ----
            outt_tiles = {}

            def emit_attn(tcp, st_lo, st_hi):
                tc0 = 2 * tcp
                if tcp not in outt_tiles:
                    oa = psum_acc.tile([65, 512], F32, tag="acc", name=f"outt_a{tcp}")
                    ob = psum_acc.tile([65, 512], F32, tag="acc", name=f"outt_b{tcp}")
                    outt_tiles[tcp] = (oa, ob)
                outt_a, outt_b = outt_tiles[tcp]
                for st in range(st_lo, st_hi):
                    kt_slice = kt_sb[:, st * 128 : (st + 1) * 128]
                    sc_ps = psum_sc.tile([128, 1024], F32, tag="sc")
                    for i in range(2):
                        nc.tensor.matmul(
                            sc_ps[:, i * 512 : (i + 1) * 512],
                            kt_slice,
                            qt_sb[:, (tc0 + i) * 512 : (tc0 + i + 1) * 512],
                            start=True,
                            stop=True,
                        )
                    ex = exp_pool.tile([128, 1024], BF16, tag="exp")
                    nc.scalar.activation(ex[:], sc_ps[:], EXP, scale=0.125)
                    for i, outt_ps in enumerate((outt_a, outt_b)):
                        nc.tensor.matmul(
                            outt_ps[:],
                            va[:, st, 0:65],
                            ex[:, i * 512 : (i + 1) * 512],
                            start=(st == 0),
                            stop=(st == NST - 1),
                        )

            def emit_epilogue(tcp):
                tc0 = 2 * tcp
                for i, outt_ps in enumerate(outt_tiles[tcp]):
                    tci = tc0 + i
                    outt_sb = outts_pool.tile([65, 512], F32, tag="outts")
                    nc.vector.tensor_copy(outt_sb[:], outt_ps[:])
                    for k in range(4):
                        o_ps = psum_p1.tile([128, 65], F32, tag="p1")
                        nc.tensor.transpose(
                            o_ps[:], outt_sb[:, k * 128 : (k + 1) * 128], ident_f32[0:65, 0:65]
                        )
                        rc = small_pool.tile([128, 1], F32, tag="rc")
                        nc.vector.reciprocal(rc[:], o_ps[:, 64:65])
                        o_sb = small_pool.tile([128, H], F32, tag="osb")
                        nc.vector.tensor_scalar_mul(o_sb[:], o_ps[:, 0:H], rc[:])
                        row = tci * 512 + k * 128
                        nc.sync.dma_start(out[row : row + 128, :], o_sb[:])

            # emission: proj blocks with attention pipelined underneath as the
            # needed K tiles / Q chunks become available.
            emit_proj_block(0)
            emit_proj_block(1)  # q chunks 0,1 ready after this
            emit_attn(0, 0, 4)
            emit_proj_block(2)
            emit_attn(0, 4, 8)
            emit_proj_block(3)
            emit_attn(0, 8, 12)
            emit_proj_block(4)
            emit_attn(0, 12, 16)
            emit_proj_block(5)
            emit_attn(0, 16, 20)
            emit_proj_block(6)
            emit_attn(0, 20, 24)
            emit_proj_block(7)
            emit_attn(0, 24, 32)
            emit_epilogue(0)
            emit_attn(1, 0, 32)
            emit_epilogue(1)

    nc.compile()
    return nc


_NC_CACHE = None


def _get_module():
    global _NC_CACHE
    if _NC_CACHE is None:
        _NC_CACHE = _build_module()
    return _NC_CACHE


def _make_in_maps(x, Wq, Wk, Wv):
    import ml_dtypes

    bf16 = ml_dtypes.bfloat16
    xT = np.transpose(np.asarray(x, dtype=np.float32), (0, 2, 1)).astype(bf16)  # [B,C,T]
    wq = np.asarray(Wq, dtype=np.float32)
    wk = np.asarray(Wk, dtype=np.float32)
    wv = np.asarray(Wv, dtype=np.float32)
    # [Wk|Wv] -> [128, 8*128]; [Wq] -> [128, 8*64]  (per-cchunk stationary)
    wkv = (
        np.concatenate([wk, wv], axis=1)  # [1024, 128]
        .reshape(NC_CH, 128, 128)
        .transpose(1, 0, 2)
        .astype(bf16)
    )
    wqh = wq.reshape(NC_CH, 128, 64).transpose(1, 0, 2).astype(bf16)
    in_maps = []
    for core in range(N_CORES):
        b, h = divmod(core, 2)
        xb = xT[b]
        if h == 1:
            xb = np.concatenate([xb[:, TQ:], xb[:, :TQ]], axis=1)
        # [C,T] -> [16 half-blocks, 128, 4, 512]
        xb = (
            xb.reshape(2, 4, 128, NSB, 512)
            .transpose(3, 0, 2, 1, 4)
            .reshape(2 * NSB, 128, 4, 512)
        )
        in_maps.append(
            {
                "xt": np.ascontiguousarray(xb),
                "wkv": np.ascontiguousarray(wkv),
                "wq": np.ascontiguousarray(wqh),
            }
        )
    return in_maps


def run(x, Wq, Wk, Wv, **spmd_kwargs):
    """Run on hardware; returns (output, BassKernelResults)."""
    nc = _get_module()
    in_maps = _make_in_maps(x, Wq, Wk, Wv)
    res = run_bass_kernel_spmd(nc, in_maps, core_ids=list(range(N_CORES)), **spmd_kwargs)
    out = np.empty((B, T, H), dtype=np.float32)
    for core in range(N_CORES):
        b, h = divmod(core, 2)
        out[b, h * TQ : (h + 1) * TQ, :] = res.results[core]["out"]
    return out, res


def kernel(x, Wq, Wk, Wv):
    out, _ = run(x, Wq, Wk, Wv)
    return out
